# revision 12
# baseline (speedup 1.0000x reference)
"""Causal attention (B=4, S=2048, D=1024, single head) on 8 TRN2 NeuronCores.

Sharding: data-parallel over batch (4 pairs of cores); within each pair
the K/V context is split by interleaved 128-row chunks (core parity p
owns global k-chunks {2j+p}).

Algebraic folding: scores = (x Wq^T)(x Wk^T)^T = x (Wq^T Wk) x^T, so the
host folds M = Wq^T Wk once (weight-only precompute) and the device
projects qm = x M for its own 1024 rows; the raw x^T it already holds
serves directly as the score lhsT (the K projection disappears).  The
pair exchanges qm halves with a 2-core AllGather so both cores hold qm
for all 2048 rows in a canonical "gathered" column order ([all even
128-blocks | all odd 128-blocks] — rank-indexed, hence identical on both
cores).  Each core then computes its causal score blocks against its own
context and produces *unnormalized* partial attention output plus the
per-row partial softmax denominator.  The host adds the two partials of
each pair and normalizes.

The SPMD program is identical across cores; all parity-dependent causal
structure lives in input data (per-core column-permuted x, per-core mask
tiles).  All matmuls run in bf16 (fp32 PSUM accumulation); inputs are
pre-cast on the host.  DMA count is kept low: each dma_start costs
~0.6us of issue time on its queue engine.
"""

import sys

if "/opt/trn_rl_repo" not in sys.path:
    sys.path.insert(0, "/opt/trn_rl_repo")

import ml_dtypes
import numpy as np

import concourse.bacc as bacc
import concourse.tile as tile
from concourse import mybir
from concourse.bass_utils import run_bass_kernel_spmd

# bass_utils imports antenv.axon_hooks when tracing is requested (e.g. via a
# BASS_TRACE env var); the image's antenv lacks that module, so provide a
# no-op fallback rather than crashing.
try:
    import antenv.axon_hooks  # noqa: F401
except ImportError:
    import types as _types

    _ah = _types.ModuleType("antenv.axon_hooks")
    _ah._hook = None
    _ah.set_axon_ntff_profile_hook = lambda h: setattr(_ah, "_hook", h)
    _ah.get_axon_ntff_profile_hook = lambda: _ah._hook
    sys.modules["antenv.axon_hooks"] = _ah

B, S, D = 4, 2048, 1024
NB = S // 128          # 16 q-blocks of 128 per batch
NT = S // 512          # 4 q-tiles of 512
IC = D // 128          # 8 contraction chunks
OC = D // 128          # 8 output-dim chunks
LC = 8                 # local k-chunks per core (S/2/128)
NMSK = 16              # mask tiles: 4 per q-tile
SCALE = 1.0 / np.sqrt(D)  # 0.03125
NJ_TILE = [4, 8, 4, 8]  # local k-chunks needed per gathered q-tile

BF16 = mybir.dt.bfloat16
F32 = mybir.dt.float32

_module_cache = None
last_results = None  # BassKernelResults of the most recent run (for test harness)


def _masked_js(tt):
    """Local chunk indices whose score blocks need a mask for q-tile tt."""
    return range(4) if tt in (0, 2) else range(4, 8)


def _build_module():
    nc = bacc.Bacc("TRN2", target_bir_lowering=False, debug=False, num_devices=8)
    # x arrives as contiguous (i, half) blocks of [128, 512]; M split into
    # contiguous per-i [128,256] + [128,768] blocks (so wave-1 DMAs read
    # contiguous DRAM at full HBM efficiency)
    xT = nc.dram_tensor("xT", [IC * 2 * 128, 512], BF16, kind="ExternalInput").ap()
    wm = nc.dram_tensor("wm", [IC * 128, D], BF16, kind="ExternalInput").ap()
    wvT = nc.dram_tensor("wvT", [D, D], BF16, kind="ExternalInput").ap()
    msk = nc.dram_tensor("msk", [NMSK * 128, 512], BF16, kind="ExternalInput").ap()
    out_p = nc.dram_tensor("out_p", [S, D], F32, kind="ExternalOutput").ap()
    rs_out = nc.dram_tensor("rs_out", [1, S], F32, kind="ExternalOutput").ap()

    with tile.TileContext(nc) as tc:
        with (
            tc.tile_pool(name="wp", bufs=1) as wp,
            tc.tile_pool(name="xp", bufs=1) as xp,
            tc.tile_pool(name="kqv", bufs=1) as kqv,
            tc.tile_pool(name="mp", bufs=1) as mp,
            tc.tile_pool(name="ptp", bufs=2) as ptp,
            tc.tile_pool(name="stg", bufs=4) as stg,
            tc.tile_pool(name="qsg", bufs=2) as qsg,
            tc.tile_pool(name="dr", bufs=1, space="DRAM") as dr,
        ):
            xt_sb = [
                xp.tile([128, S // 2], BF16, tag=f"x{i}", name=f"x{i}")
                for i in range(IC)
            ]
            wm_sb = [
                wp.tile([128, D], BF16, tag=f"wm{i}", name=f"wm{i}") for i in range(IC)
            ]
            wv_sb = [
                wp.tile([128, D], BF16, tag=f"wv{i}", name=f"wv{i}") for i in range(IC)
            ]
            # wave 1: x first half + M (one whole-tile DMA per chunk so the
            # i=0 projection chain unblocks after a single transfer)
            for i in range(IC):
                nc.sync.dma_start(
                    xt_sb[i][:, 0:512], xT[128 * 2 * i : 128 * (2 * i + 1), :]
                )
            for i in range(IC):
                nc.scalar.dma_start(wm_sb[i], wm[128 * i : 128 * (i + 1), :])
            # wave 2: x second half + Wv
            for i in range(IC):
                nc.sync.dma_start(
                    xt_sb[i][:, 512:1024], xT[128 * (2 * i + 1) : 128 * (2 * i + 2), :]
                )
            for i in range(IC):
                nc.scalar.dma_start(wv_sb[i], wvT[128 * i : 128 * (i + 1), :])
            # wave 3: masks (one strided DMA for all 16 mask tiles)
            mask_all = mp.tile([128, NMSK, 512], BF16, tag="masks", name="masks")
            nc.scalar.dma_start(mask_all, msk.rearrange("(m p) c -> p m c", p=128))
            ones_sb = mp.tile([128, 1], BF16, tag="ones", name="ones")
            nc.any.memset(ones_sb, 1.0)

            qt_all = kqv.tile([128, OC, S], BF16, tag="qt", name="qt")
            vn_sb = [kqv.tile([128, D], BF16, tag=f"vn{j}", name=f"vn{j}") for j in range(LC)]

            # DRAM bounce buffers for the pairwise qm-half exchange (split in
            # two st-pieces so the exchange pipelines with the projections).
            # Separate Internal tensors (not pool tiles) so the two pieces
            # carry no false dependencies on each other; Shared addr space on
            # the gather outputs is the fast path for HBM-HBM collectives.
            qhalf = [
                nc.dram_tensor(f"qhalf{st}", [D, 512], BF16, kind="Internal").ap()
                for st in range(2)
            ]
            qfull = [
                nc.dram_tensor(f"qfull{st}", [2 * D, 512], BF16, kind="Internal").ap()
                for st in range(2)
            ]

            # ---- phase 1 projections: i-outer chains across 8 PSUM banks so
            #      the PE starts as soon as the first (w, x) chunks land ----
            def proj_iouter(ps1, lhs_slices, rhs_slices, dsts, pname):
                pps = [
                    ps1.tile([128, 512], F32, tag=f"proj8_{o}", bufs=1, name=f"{pname}{o}")
                    for o in range(len(dsts))
                ]
                for i in range(IC):
                    for o in range(len(dsts)):
                        nc.tensor.matmul(
                            pps[o],
                            lhsT=lhs_slices(i, o),
                            rhs=rhs_slices(i, o),
                            start=(i == 0),
                            stop=(i == IC - 1),
                        )
                for o, dst in enumerate(dsts):
                    dst(pps[o])

            def copy_to(dst, o=0):
                # phase-1 copies alternate DVE / Scalar so the 8 PSUM->SBUF
                # casts of a projection wave drain in ~half the serial time
                # (GpSimd cannot read PSUM)
                if o % 2 == 0:
                    return lambda pp: nc.vector.tensor_copy(dst, pp)
                return lambda pp: nc.scalar.copy(dst, pp)

            def q_own_phase(ps1, st):
                """Project this core's own qm half (local even blocks of x),
                stage to SBUF, then one batched DMA into qhalf[st]."""
                qs = qsg.tile([128, OC, 512], BF16, tag="qsg", name="qsg")
                proj_iouter(
                    ps1,
                    lambda i, o: wm_sb[i][:, 128 * o : 128 * (o + 1)],
                    lambda i, o: xt_sb[i][:, 512 * st : 512 * (st + 1)],
                    [copy_to(qs[:, o, :], o) for o in range(OC)],
                    f"pq{st}",
                )
                nc.sync.dma_start(
                    qhalf[st].rearrange("(o p) c -> p o c", p=128), qs
                )

            def q_exchange(st):
                nc.gpsimd.collective_compute(
                    kind="AllGather",
                    op=mybir.AluOpType.bypass,
                    replica_groups=[[0, 1], [2, 3], [4, 5], [6, 7]],
                    ins=[qhalf[st]],
                    outs=[qfull[st]],
                )
                for r in range(2):
                    nc.sync.dma_start(
                        qt_all[:, :, 1024 * r + 512 * st : 1024 * r + 512 * (st + 1)],
                        qfull[st][1024 * r : 1024 * (r + 1), :].rearrange(
                            "(o p) c -> p o c", p=128
                        ),
                    )

            with tc.tile_pool(name="ps1", bufs=1, space="PSUM") as ps1:
                q_own_phase(ps1, 0)
                q_exchange(0)
                q_own_phase(ps1, 1)
                q_exchange(1)
                # V projection for chunks j<4 (all that attention tiles 0 and
                # 2 need); chunks j>=4 are projected later, between attention
                # tiles 2 and 1, to cover the latency of the second exchange
                proj_iouter(
                    ps1,
                    lambda i, c: xt_sb[i][:, 128 * (c // 2) : 128 * (c // 2 + 1)],
                    lambda i, c: wv_sb[i][:, 512 * (c % 2) : 512 * (c % 2 + 1)],
                    [
                        (lambda dst: lambda pp: nc.any.tensor_copy(dst, pp))(
                            vn_sb[c // 2][:, 512 * (c % 2) : 512 * (c % 2 + 1)]
                        )
                        for c in range(8)
                    ],
                    "pva",
                )

            # ---- phase 2: attention over gathered q-tiles; tiles 0 and 2
            #      only depend on the first exchange piece.  The raw x^T in
            #      SBUF is the score lhsT (k == x after the M folding). ----
            rs_sb = mp.tile([1, S], F32, tag="rs", name="rs")
            with tc.tile_pool(name="ps2", bufs=2, space="PSUM") as ps:

                def vn_late():
                    for c in range(8):
                        j, ot = 4 + c // 2, c % 2
                        pp = ps.tile([128, 512], F32, tag="score", bufs=4, name="pvb")
                        for i in range(IC):
                            nc.tensor.matmul(
                                pp,
                                lhsT=xt_sb[i][:, 128 * j : 128 * (j + 1)],
                                rhs=wv_sb[i][:, 512 * ot : 512 * (ot + 1)],
                                start=(i == 0),
                                stop=(i == IC - 1),
                            )
                        if c % 2 == 0:
                            nc.vector.tensor_copy(
                                vn_sb[j][:, 512 * ot : 512 * (ot + 1)], pp
                            )
                        else:
                            nc.scalar.copy(
                                vn_sb[j][:, 512 * ot : 512 * (ot + 1)], pp
                            )

                def attention_tile(tt):
                    nj = NJ_TILE[tt]
                    masked = set(_masked_js(tt))
                    pt_tiles = []
                    offs = []
                    for j in range(nj):
                        # in a masked (diagonal-region) block, the first
                        # 128*(j%4) gathered q-columns are fully masked out —
                        # skip computing them entirely
                        off = 128 * (j % 4) if j in masked else 0
                        offs.append(off)
                        sp = ps.tile([128, 512], F32, tag="score", bufs=4, name="score")
                        for o in range(OC):
                            nc.tensor.matmul(
                                sp[:, off:512],
                                lhsT=xt_sb[o][:, 128 * j : 128 * (j + 1)],
                                rhs=qt_all[:, o, 512 * tt + off : 512 * (tt + 1)],
                                start=(o == 0),
                                stop=(o == OC - 1),
                            )
                        pt = ptp.tile([128, 512], BF16, tag=f"pt{j}", name=f"pt{j}")
                        nc.scalar.activation(
                            pt[:, off:512],
                            sp[:, off:512],
                            mybir.ActivationFunctionType.Exp,
                            scale=SCALE,
                        )
                        if j in masked:
                            m = 4 * tt + (j % 4)
                            nc.vector.tensor_mul(
                                pt[:, off:512], pt[:, off:512], mask_all[:, m, off:512]
                            )
                        pt_tiles.append(pt)

                    # partial softmax denominators: ones^T @ pt accumulated over j
                    rsp = ps.tile([1, 512], F32, tag="rs", bufs=1, name="rsp")
                    for j in range(nj):
                        nc.tensor.matmul(
                            rsp[:, offs[j] : 512],
                            lhsT=ones_sb,
                            rhs=pt_tiles[j][:, offs[j] : 512],
                            start=(j == 0),
                            stop=(j == nj - 1),
                        )
                    nc.vector.tensor_copy(rs_sb[:, 512 * tt : 512 * (tt + 1)], rsp)

                    for qq in (3, 2, 1, 0):
                        qbg = 4 * tt + qq        # gathered q-block index
                        njs = (qbg % 8) + 1      # causal chunk count in gathered order
                        ost = stg.tile([128, D], F32, tag="ost", name="ost")
                        for ot in range(2):
                            apsum = ps.tile(
                                [128, 512], F32, tag="attn", bufs=3, name="attn"
                            )
                            for j in range(njs):
                                nc.tensor.matmul(
                                    apsum,
                                    lhsT=pt_tiles[j][:, 128 * qq : 128 * (qq + 1)],
                                    rhs=vn_sb[j][:, 512 * ot : 512 * (ot + 1)],
                                    start=(j == 0),
                                    stop=(j == njs - 1),
                                )
                            nc.vector.tensor_copy(ost[:, 512 * ot : 512 * (ot + 1)], apsum)
                        # alternate store queues so the last tile's output
                        # drains on two DMA rings concurrently
                        eng = nc.gpsimd if qq % 2 == 0 else nc.sync
                        eng.dma_start(
                            out_p[128 * qbg : 128 * (qbg + 1), :], ost
                        )
                # vn_late first: it only needs local x/Wv, adding ~14us of PE
                # cover in front of tile 0's dependency on exchange piece 0
                vn_late()
                attention_tile(0)
                attention_tile(2)
                attention_tile(1)
                attention_tile(3)

            nc.gpsimd.dma_start(rs_out, rs_sb)

    nc.compile()
    return nc


def _get_module():
    global _module_cache
    if _module_cache is None:
        _module_cache = _build_module()
    return _module_cache


def _gathered_q(p):
    """Global q index for gathered position p (vectorized)."""
    p = np.asarray(p)
    blk = p // 128
    even = blk < 8
    gb = np.where(even, 2 * blk, 2 * (blk - 8) + 1)
    return 128 * gb + p % 128


def _host_masks(par: int) -> np.ndarray:
    """[NMSK*128, 512] bf16 causal masks in gathered q order."""
    out = np.zeros((NMSK * 128, 512), dtype=np.float32)
    k = np.arange(128)[:, None]
    ql = np.arange(512)[None, :]
    for tt in range(NT):
        for idx, j in enumerate(_masked_js(tt)):
            m = 4 * tt + idx
            g = 2 * j + par  # global k-chunk of local chunk j
            q_global = _gathered_q(512 * tt + ql)
            out[128 * m : 128 * (m + 1), :] = (q_global >= 128 * g + k).astype(
                np.float32
            )
    return out.astype(ml_dtypes.bfloat16)


def kernel(x, Wq, Wk, Wv, _trace=False):
    global last_results
    nc = _get_module()

    bf = ml_dtypes.bfloat16

    # weight-only folding: scores = x (Wq^T Wk) x^T
    wm = np.ascontiguousarray((Wq.T @ Wk).astype(bf))
    wvT = np.ascontiguousarray(Wv.T).astype(bf)
    masks = [_host_masks(0), _host_masks(1)]

    # per-parity column selection: core owns global k-chunks {2j+par}
    own_cols = [
        (128 * (2 * np.arange(LC)[:, None] + par) + np.arange(128)[None, :]).reshape(-1)
        for par in range(2)
    ]

    in_maps = []
    for c in range(8):
        b, par = c // 2, c % 2
        xTb = x[b].T[:, own_cols[par]].astype(bf)  # [D, S//2]
        # pack as contiguous (i, half) blocks of [128, 512]
        xpk = np.ascontiguousarray(
            xTb.reshape(IC, 128, 2, 512).transpose(0, 2, 1, 3)
        ).reshape(IC * 2 * 128, 512)
        in_maps.append(
            {
                "xT": xpk,
                "wm": wm,
                "wvT": wvT,
                "msk": masks[par],
            }
        )

    kwargs = {}
    if _trace:
        kwargs["trace"] = True
    res = run_bass_kernel_spmd(nc, in_maps, core_ids=list(range(8)), **kwargs)
    last_results = res

    # rows come back in gathered order; gath_row[q] = gathered position of q
    gath_row = np.empty(S, dtype=np.int64)
    gath_row[_gathered_q(np.arange(S))] = np.arange(S)

    out = np.empty((B, S, D), dtype=np.float32)
    for b in range(B):
        rA = res.results[2 * b]
        rB = res.results[2 * b + 1]
        num = rA["out_p"] + rB["out_p"]
        den = rA["rs_out"][0] + rB["rs_out"][0]
        out[b] = (num / den[:, None])[gath_row]
    return out


# revision 21
# speedup vs baseline: 1.0957x; 1.0957x over previous
"""Causal attention (B=4, S=2048, D=1024, single head) on 8 TRN2 NeuronCores.

Sharding: data-parallel over batch (4 pairs of cores); within each pair
the K/V context is split by interleaved 128-row chunks (core parity p
owns global k-chunks {2j+p}).

Algebraic folding: scores = (x Wq^T)(x Wk^T)^T = x (Wq^T Wk) x^T, so the
host folds M = Wq^T Wk once (weight-only precompute) and the device
projects qm = x M for its own 1024 rows; the raw x^T it already holds
serves directly as the score lhsT (the K projection disappears).  The
pair exchanges qm halves with a 2-core AllGather so both cores hold qm
for all 2048 rows in a canonical "gathered" column order ([all even
128-blocks | all odd 128-blocks] — rank-indexed, hence identical on both
cores).  Each core then computes its causal score blocks against its own
context and produces *unnormalized* partial attention output plus the
per-row partial softmax denominator.  The host adds the two partials of
each pair and normalizes.

The SPMD program is identical across cores; all parity-dependent causal
structure lives in input data (per-core column-permuted x, per-core mask
tiles).  All matmuls run in bf16 (fp32 PSUM accumulation); inputs are
pre-cast on the host.  DMA count is kept low: each dma_start costs
~0.6us of issue time on its queue engine.
"""

import sys

if "/opt/trn_rl_repo" not in sys.path:
    sys.path.insert(0, "/opt/trn_rl_repo")

import ml_dtypes
import numpy as np

import concourse.bacc as bacc
import concourse.tile as tile
from concourse import mybir
from concourse.bass_utils import run_bass_kernel_spmd

# bass_utils imports antenv.axon_hooks when tracing is requested (e.g. via a
# BASS_TRACE env var); the image's antenv lacks that module, so provide a
# no-op fallback rather than crashing.
try:
    import antenv.axon_hooks  # noqa: F401
except ImportError:
    import types as _types

    _ah = _types.ModuleType("antenv.axon_hooks")
    _ah._hook = None
    _ah.set_axon_ntff_profile_hook = lambda h: setattr(_ah, "_hook", h)
    _ah.get_axon_ntff_profile_hook = lambda: _ah._hook
    sys.modules["antenv.axon_hooks"] = _ah

B, S, D = 4, 2048, 1024
NB = S // 128          # 16 q-blocks of 128 per batch
NT = S // 512          # 4 q-tiles of 512
IC = D // 128          # 8 contraction chunks
OC = D // 128          # 8 output-dim chunks
LC = 8                 # local k-chunks per core (S/2/128)
NMSK = 16              # mask tiles: 4 per q-tile
SCALE = 1.0 / np.sqrt(D)  # 0.03125
NJ_TILE = [4, 8, 4, 8]  # local k-chunks needed per gathered q-tile

BF16 = mybir.dt.bfloat16
F32 = mybir.dt.float32

_module_cache = None
last_results = None  # BassKernelResults of the most recent run (for test harness)


def _masked_js(tt):
    """Local chunk indices whose score blocks need a mask for q-tile tt."""
    return range(4) if tt in (0, 2) else range(4, 8)


def _build_module():
    nc = bacc.Bacc("TRN2", target_bir_lowering=False, debug=False, num_devices=8)
    # x arrives as contiguous (i, half) blocks of [128, 512]; M split into
    # contiguous per-i [128,256] + [128,768] blocks (so wave-1 DMAs read
    # contiguous DRAM at full HBM efficiency)
    xT = nc.dram_tensor("xT", [IC * 2 * 128, 512], BF16, kind="ExternalInput").ap()
    wm = nc.dram_tensor("wm", [IC * 128, D], BF16, kind="ExternalInput").ap()
    wvT = nc.dram_tensor("wvT", [D, D], BF16, kind="ExternalInput").ap()
    msk = nc.dram_tensor("msk", [NMSK * 128, 512], BF16, kind="ExternalInput").ap()
    out_p = nc.dram_tensor("out_p", [S, D], F32, kind="ExternalOutput").ap()
    rs_out = nc.dram_tensor("rs_out", [1, S], F32, kind="ExternalOutput").ap()

    with tile.TileContext(nc) as tc:
        with (
            tc.tile_pool(name="wp", bufs=1) as wp,
            tc.tile_pool(name="xp", bufs=1) as xp,
            tc.tile_pool(name="kqv", bufs=1) as kqv,
            tc.tile_pool(name="mp", bufs=1) as mp,
            tc.tile_pool(name="ptp", bufs=2) as ptp,
            tc.tile_pool(name="stg", bufs=4) as stg,
            tc.tile_pool(name="qsg", bufs=2) as qsg,
            tc.tile_pool(name="dr", bufs=1, space="DRAM") as dr,
        ):
            xt_sb = [
                xp.tile([128, S // 2], BF16, tag=f"x{i}", name=f"x{i}")
                for i in range(IC)
            ]
            wm_sb = [
                wp.tile([128, D], BF16, tag=f"wm{i}", name=f"wm{i}") for i in range(IC)
            ]
            wv_sb = [
                wp.tile([128, D], BF16, tag=f"wv{i}", name=f"wv{i}") for i in range(IC)
            ]
            # wave 1: x first half + M (one whole-tile DMA per chunk so the
            # i=0 projection chain unblocks after a single transfer)
            for i in range(IC):
                nc.sync.dma_start(
                    xt_sb[i][:, 0:512], xT[128 * 2 * i : 128 * (2 * i + 1), :]
                )
            for i in range(IC):
                nc.scalar.dma_start(wm_sb[i], wm[128 * i : 128 * (i + 1), :])
            # wave 2: x second half + Wv
            for i in range(IC):
                nc.sync.dma_start(
                    xt_sb[i][:, 512:1024], xT[128 * (2 * i + 1) : 128 * (2 * i + 2), :]
                )
            for i in range(IC):
                nc.scalar.dma_start(wv_sb[i], wvT[128 * i : 128 * (i + 1), :])
            # wave 3: masks (one strided DMA for all 16 mask tiles)
            mask_all = mp.tile([128, NMSK, 512], BF16, tag="masks", name="masks")
            nc.scalar.dma_start(mask_all, msk.rearrange("(m p) c -> p m c", p=128))
            ones_sb = mp.tile([128, 1], BF16, tag="ones", name="ones")
            nc.any.memset(ones_sb, 1.0)

            # gathered qm, indexed [o%2 (even/odd), o//2, col] so the even- and
            # odd-o halves are separately DMA-able without strided APs
            qt_all = kqv.tile([128, 2, OC // 2, S], BF16, tag="qt", name="qt")
            vn_sb = [kqv.tile([128, D], BF16, tag=f"vn{j}", name=f"vn{j}") for j in range(LC)]

            # DRAM bounce buffers for the pairwise qm-half exchange (split in
            # two st-pieces so the exchange pipelines with the projections).
            # Separate Internal tensors (not pool tiles) so the two pieces
            # carry no false dependencies on each other.  Row layout per half:
            # [4 even-o blocks | 4 odd-o blocks] x 128 rows.
            qhalf = [
                nc.dram_tensor(f"qhalf{st}", [D, 512], BF16, kind="Internal").ap()
                for st in range(2)
            ]
            qfull = [
                nc.dram_tensor(f"qfull{st}", [2 * D, 512], BF16, kind="Internal").ap()
                for st in range(2)
            ]

            # ---- phase 1 projections: i-outer chains across 8 PSUM banks so
            #      the PE starts as soon as the first (w, x) chunks land ----
            def proj_iouter(ps1, lhs_slices, rhs_slices, dsts, pname):
                pps = [
                    ps1.tile([128, 512], F32, tag=f"proj8_{o}", bufs=1, name=f"{pname}{o}")
                    for o in range(len(dsts))
                ]
                for i in range(IC):
                    for o in range(len(dsts)):
                        nc.tensor.matmul(
                            pps[o],
                            lhsT=lhs_slices(i, o),
                            rhs=rhs_slices(i, o),
                            start=(i == 0),
                            stop=(i == IC - 1),
                        )
                for o, dst in enumerate(dsts):
                    dst(pps[o])

            def copy_to(dst, o=0):
                # phase-1 copies alternate DVE / Scalar so the 8 PSUM->SBUF
                # casts of a projection wave drain in ~half the serial time
                # (GpSimd cannot read PSUM)
                if o % 2 == 0:
                    return lambda pp: nc.vector.tensor_copy(dst, pp)
                return lambda pp: nc.scalar.copy(dst, pp)

            def q_own_phase(ps1, st):
                """Project this core's own qm half (local even blocks of x),
                stage to SBUF in even/odd-o halves, then DMA each half into
                qhalf[st] as soon as its four casts land.  Stores for piece 1
                go on the opposite queues from piece 0's loads so a store is
                never queued behind an AllGather-gated load."""
                qsE = qsg.tile([128, 4, 512], BF16, tag="qsgE", name="qsgE")
                qsO = qsg.tile([128, 4, 512], BF16, tag="qsgO", name="qsgO")
                proj_iouter(
                    ps1,
                    lambda i, o: wm_sb[i][:, 128 * o : 128 * (o + 1)],
                    lambda i, o: xt_sb[i][:, 512 * st : 512 * (st + 1)],
                    [
                        copy_to((qsE if o % 2 == 0 else qsO)[:, o // 2, :], o)
                        for o in range(OC)
                    ],
                    f"pq{st}",
                )
                nc.sync.dma_start(
                    qhalf[st][0:512, :].rearrange("(o p) c -> p o c", p=128), qsE
                )
                nc.scalar.dma_start(
                    qhalf[st][512:1024, :].rearrange("(o p) c -> p o c", p=128), qsO
                )

            def q_exchange(st):
                nc.gpsimd.collective_compute(
                    kind="AllGather",
                    op=mybir.AluOpType.bypass,
                    replica_groups=[[0, 1], [2, 3], [4, 5], [6, 7]],
                    ins=[qhalf[st]],
                    outs=[qfull[st]],
                )
                for r in range(2):
                    for h in range(2):
                        nc.sync.dma_start(
                            qt_all[
                                :, h, :,
                                1024 * r + 512 * st : 1024 * r + 512 * (st + 1),
                            ],
                            qfull[st][
                                1024 * r + 512 * h : 1024 * r + 512 * (h + 1), :
                            ].rearrange("(o p) c -> p o c", p=128),
                        )

            with tc.tile_pool(name="ps1", bufs=1, space="PSUM") as ps1:
                # stores (sync/scalar) are enqueued before any AllGather-gated
                # qt load hits a ring; gpsimd carries only the cc triggers,
                # which are semaphore-gated, not ring-position-gated
                q_own_phase(ps1, 0)
                q_own_phase(ps1, 1)
                q_exchange(0)
                q_exchange(1)
                # V projection for chunks j<4 (all that attention tiles 0 and
                # 2 need); chunks j>=4 are projected later, between attention
                # tiles 2 and 1, to cover the latency of the second exchange
                proj_iouter(
                    ps1,
                    lambda i, c: xt_sb[i][:, 128 * (c // 2) : 128 * (c // 2 + 1)],
                    lambda i, c: wv_sb[i][:, 512 * (c % 2) : 512 * (c % 2 + 1)],
                    [
                        (lambda dst: lambda pp: nc.any.tensor_copy(dst, pp))(
                            vn_sb[c // 2][:, 512 * (c % 2) : 512 * (c % 2 + 1)]
                        )
                        for c in range(8)
                    ],
                    "pva",
                )

            # ---- phase 2: attention over gathered q-tiles; tiles 0 and 2
            #      only depend on the first exchange piece.  The raw x^T in
            #      SBUF is the score lhsT (k == x after the M folding). ----
            rs_sb = mp.tile([1, S], F32, tag="rs", name="rs")
            with tc.tile_pool(name="ps2", bufs=2, space="PSUM") as ps:

                def vn_late():
                    for c in range(8):
                        j, ot = 4 + c // 2, c % 2
                        pp = ps.tile([128, 512], F32, tag="score", bufs=4, name="pvb")
                        for i in range(IC):
                            nc.tensor.matmul(
                                pp,
                                lhsT=xt_sb[i][:, 128 * j : 128 * (j + 1)],
                                rhs=wv_sb[i][:, 512 * ot : 512 * (ot + 1)],
                                start=(i == 0),
                                stop=(i == IC - 1),
                            )
                        nc.vector.tensor_copy(
                            vn_sb[j][:, 512 * ot : 512 * (ot + 1)], pp
                        )

                def attention_tile(tt):
                    nj = NJ_TILE[tt]
                    masked = set(_masked_js(tt))
                    pt_tiles = []
                    offs = []
                    for j in range(nj):
                        # in a masked (diagonal-region) block, the first
                        # 128*(j%4) gathered q-columns are fully masked out —
                        # skip computing them entirely
                        off = 128 * (j % 4) if j in masked else 0
                        offs.append(off)
                        sp = ps.tile([128, 512], F32, tag="score", bufs=4, name="score")
                        for o in range(OC):
                            nc.tensor.matmul(
                                sp[:, off:512],
                                lhsT=xt_sb[o][:, 128 * j : 128 * (j + 1)],
                                rhs=qt_all[
                                    :, o % 2, o // 2,
                                    512 * tt + off : 512 * (tt + 1),
                                ],
                                start=(o == 0),
                                stop=(o == OC - 1),
                            )
                        pt = ptp.tile([128, 512], BF16, tag=f"pt{j}", name=f"pt{j}")
                        nc.scalar.activation(
                            pt[:, off:512],
                            sp[:, off:512],
                            mybir.ActivationFunctionType.Exp,
                            scale=SCALE,
                        )
                        if j in masked:
                            m = 4 * tt + (j % 4)
                            nc.vector.tensor_mul(
                                pt[:, off:512], pt[:, off:512], mask_all[:, m, off:512]
                            )
                        pt_tiles.append(pt)

                    # partial softmax denominators: ones^T @ pt accumulated over j
                    rsp = ps.tile([1, 512], F32, tag="rs", bufs=1, name="rsp")
                    for j in range(nj):
                        nc.tensor.matmul(
                            rsp[:, offs[j] : 512],
                            lhsT=ones_sb,
                            rhs=pt_tiles[j][:, offs[j] : 512],
                            start=(j == 0),
                            stop=(j == nj - 1),
                        )
                    nc.vector.tensor_copy(rs_sb[:, 512 * tt : 512 * (tt + 1)], rsp)

                    for qq in (3, 2, 1, 0):
                        qbg = 4 * tt + qq        # gathered q-block index
                        njs = (qbg % 8) + 1      # causal chunk count in gathered order
                        ost = stg.tile([128, D], F32, tag="ost", name="ost")
                        for ot in range(2):
                            apsum = ps.tile(
                                [128, 512], F32, tag="attn", bufs=3, name="attn"
                            )
                            for j in range(njs):
                                nc.tensor.matmul(
                                    apsum,
                                    lhsT=pt_tiles[j][:, 128 * qq : 128 * (qq + 1)],
                                    rhs=vn_sb[j][:, 512 * ot : 512 * (ot + 1)],
                                    start=(j == 0),
                                    stop=(j == njs - 1),
                                )
                            nc.vector.tensor_copy(ost[:, 512 * ot : 512 * (ot + 1)], apsum)
                        # alternate store queues (gpsimd/scalar) so output
                        # drains on two DMA rings and the sync/vector rings
                        # stay clear for the exchange traffic
                        eng = nc.gpsimd if qq % 2 == 0 else nc.scalar
                        eng.dma_start(
                            out_p[128 * qbg : 128 * (qbg + 1), :], ost
                        )
                # vn_late first: it only needs local x/Wv, adding ~14us of PE
                # cover in front of tile 0's dependency on exchange piece 0
                vn_late()
                attention_tile(0)
                attention_tile(2)
                attention_tile(1)
                attention_tile(3)

            nc.gpsimd.dma_start(rs_out, rs_sb)

    nc.compile()
    return nc


def _get_module():
    global _module_cache
    if _module_cache is None:
        _module_cache = _build_module()
    return _module_cache


def _gathered_q(p):
    """Global q index for gathered position p (vectorized)."""
    p = np.asarray(p)
    blk = p // 128
    even = blk < 8
    gb = np.where(even, 2 * blk, 2 * (blk - 8) + 1)
    return 128 * gb + p % 128


def _host_masks(par: int) -> np.ndarray:
    """[NMSK*128, 512] bf16 causal masks in gathered q order."""
    out = np.zeros((NMSK * 128, 512), dtype=np.float32)
    k = np.arange(128)[:, None]
    ql = np.arange(512)[None, :]
    for tt in range(NT):
        for idx, j in enumerate(_masked_js(tt)):
            m = 4 * tt + idx
            g = 2 * j + par  # global k-chunk of local chunk j
            q_global = _gathered_q(512 * tt + ql)
            out[128 * m : 128 * (m + 1), :] = (q_global >= 128 * g + k).astype(
                np.float32
            )
    return out.astype(ml_dtypes.bfloat16)


def kernel(x, Wq, Wk, Wv, _trace=False):
    global last_results
    nc = _get_module()

    bf = ml_dtypes.bfloat16

    # weight-only folding: scores = x (Wq^T Wk) x^T
    wm = np.ascontiguousarray((Wq.T @ Wk).astype(bf))
    wvT = np.ascontiguousarray(Wv.T).astype(bf)
    masks = [_host_masks(0), _host_masks(1)]

    # per-parity column selection: core owns global k-chunks {2j+par}
    own_cols = [
        (128 * (2 * np.arange(LC)[:, None] + par) + np.arange(128)[None, :]).reshape(-1)
        for par in range(2)
    ]

    in_maps = []
    for c in range(8):
        b, par = c // 2, c % 2
        xTb = x[b].T[:, own_cols[par]].astype(bf)  # [D, S//2]
        # pack as contiguous (i, half) blocks of [128, 512]
        xpk = np.ascontiguousarray(
            xTb.reshape(IC, 128, 2, 512).transpose(0, 2, 1, 3)
        ).reshape(IC * 2 * 128, 512)
        in_maps.append(
            {
                "xT": xpk,
                "wm": wm,
                "wvT": wvT,
                "msk": masks[par],
            }
        )

    kwargs = {}
    if _trace:
        kwargs["trace"] = True
    res = run_bass_kernel_spmd(nc, in_maps, core_ids=list(range(8)), **kwargs)
    last_results = res

    # rows come back in gathered order; gath_row[q] = gathered position of q
    gath_row = np.empty(S, dtype=np.int64)
    gath_row[_gathered_q(np.arange(S))] = np.arange(S)

    out = np.empty((B, S, D), dtype=np.float32)
    for b in range(B):
        rA = res.results[2 * b]
        rB = res.results[2 * b + 1]
        num = rA["out_p"] + rB["out_p"]
        den = rA["rs_out"][0] + rB["rs_out"][0]
        out[b] = (num / den[:, None])[gath_row]
    return out
